# revision 26
# baseline (speedup 1.0000x reference)
"""Trainium2 Bass kernel for AspectNeighborAttention (gnn_message_passing).

Pure data-parallel over batch: 32 batches -> 8 NeuronCores x 4 batches.
All weights replicated, host-converted to bf16 and host-PRE-TRANSPOSED into
the chunk-major [128, KC, *] lhsT/rhs layouts the TensorEngine wants.

Key tricks vs the naive formulation:
 - dep is host-scaled by wa_e and stored bf16 ("dep'"); the matching 1/wa_e
   is folded into Wf[:, H:]. s_e then needs only a reduce over e, and the
   D-path result D' = sum_j attn*dep' feeds the rescaled weights.
 - weight folding: temp = nbr @ WhN^T + zs @ WhZ^T with
   nbr = attn @ (zs @ WfZ^T) + D @ WfE^T collapses via host-precomputed
   M1 = WfZ^T WhN^T and M2 = WfE'^T WhN^T to
   temp = attn @ (zs @ M1) + D' @ M2 + zs @ WhZ^T  (no nbr intermediate).
 - the big e/j reductions run as bf16 tensor_tensor ADD trees (2x DVE mode)
   with a short tensor_reduce tail; monolithic reduces get no 2x.
 - the attn * dep' multiply runs entirely on GpSimd (Pool), balancing DVE.
 - leaky-relu is computed as max(x, 0.01*x) on DVE so the ACT engine keeps
   only the Exp table loaded (no per-batch activation-table swaps).
 - input DMAs are split across the SP and Activation HWDGE queues.
 - the batch loop is software-pipelined: emission order B(b+1), C(b+1),
   D(b) keeps every engine's in-order queue free of cross-batch stalls
   (B = scores/softmax, C = Pool multiply, D = D-tree + output matmuls).

The roll(z,-1)/roll(out,+1) pair is handled purely with shifted-row DMAs.
"""

import sys

for _p in ("/opt/trn_rl_repo",):
    if _p not in sys.path:
        sys.path.insert(0, _p)

import os
import numpy as np
import ml_dtypes

import concourse.bass as bass
import concourse.bacc as bacc_mod
import concourse.mybir as mybir
import concourse.tile as tile
from concourse.masks import make_identity

B, L, H, E = 32, 128, 768, 64
NCORES = 8
PB = B // NCORES  # batches per core
KC = H // 128     # 6 k-chunks
F32 = mybir.dt.float32
BF16 = mybir.dt.bfloat16
AF = mybir.ActivationFunctionType
OP = mybir.AluOpType
AX = mybir.AxisListType
MASK_SHIFT = 10000.0  # additive mask offset (see score masking)

_CACHED = {}

CFG = dict(
    dep_bufs=int(os.environ.get("K_DEP_BUFS", 3)),
    ttmp_bufs=int(os.environ.get("K_TTMP_BUFS", 2)),
    se_bufs=int(os.environ.get("K_SE_BUFS", 1)),
    dt_bufs=int(os.environ.get("K_DT_BUFS", 1)),
    spool_bufs=int(os.environ.get("K_SPOOL_BUFS", 2)),
    opool_bufs=int(os.environ.get("K_OPOOL_BUFS", 2)),
)


def _build(debug=False):
    nc = bacc_mod.Bacc("TRN2", target_bir_lowering=False, debug=False,
                       num_devices=NCORES)

    bertS_d = nc.dram_tensor("bertS_d", [PB, L, H], F32,
                             kind="ExternalInput")
    bertsT = nc.dram_tensor("bertsT", [PB, 128, KC, 128], BF16,
                            kind="ExternalInput")
    dep = nc.dram_tensor("dep", [PB, L, L, E], BF16, kind="ExternalInput")
    adjf = nc.dram_tensor("adjf", [PB, L, L], F32, kind="ExternalInput")
    updcol = nc.dram_tensor("updcol", [PB, 128, 1], F32,
                              kind="ExternalInput")
    wzT_d = nc.dram_tensor("wzT", [128, KC, H], BF16, kind="ExternalInput")
    m1T_d = nc.dram_tensor("m1T", [128, KC, H], BF16, kind="ExternalInput")
    whzT_d = nc.dram_tensor("whzT", [128, KC, H], BF16, kind="ExternalInput")
    m2_d = nc.dram_tensor("m2", [E, H], BF16, kind="ExternalInput")
    w2T_d = nc.dram_tensor("w2T", [128, KC, 2], BF16, kind="ExternalInput")
    bzt = nc.dram_tensor("bzt", [1, H], BF16, kind="ExternalInput")
    bat = nc.dram_tensor("bat", [1, 1], F32, kind="ExternalInput")
    out = nc.dram_tensor("out", [PB, L, H], F32, kind="ExternalOutput")

    dbg = {}
    if debug:
        for nm, shape, dt in [
            ("d_zsT", [128, KC, 128], BF16), ("d_si", [1, 128], F32),
            ("d_sjb", [1, 128], F32), ("d_se", [128, L], F32),
            ("d_masked", [128, L], F32), ("d_attn", [128, L], BF16),
            ("d_dvec", [128, E], BF16), ("d_gb", [128, H], BF16),
            ("d_tempb", [128, H], F32),
            ("d_upd", [128, 1], F32), ("d_scb", [128, 128], F32),
        ]:
            dbg[nm] = nc.dram_tensor(nm, shape, dt, kind="ExternalOutput")
    with tile.TileContext(nc) as tc:
        with nc.allow_low_precision("bf16 softmax/D path, 2e-2 rel-err gate"):
            _body(tc, nc, bertS_d, bertsT, dep, adjf, updcol, wzT_d, m1T_d,
                  whzT_d, m2_d, w2T_d, bzt, bat, out, dbg)
    nc.compile()
    return nc


def _body(tc, nc, bertS_d, bertsT, dep, adjf, updcol, wzT_d, m1T_d,
          whzT_d, m2_d, w2T_d, bzt, bat, out, dbg=None):
    def dump(name, ap):
        if dbg and name in dbg:
            nc.sync.dma_start(dbg[name][...], ap)
    import contextlib
    cfg = CFG
    ctx = contextlib.ExitStack()
    with ctx:
        wpool = ctx.enter_context(tc.tile_pool(name="weights", bufs=1))
        dpool = ctx.enter_context(
            tc.tile_pool(name="dep", bufs=cfg["dep_bufs"]))
        tpool = ctx.enter_context(
            tc.tile_pool(name="ttmp", bufs=cfg["ttmp_bufs"]))
        sepool = ctx.enter_context(
            tc.tile_pool(name="setree", bufs=cfg["se_bufs"]))
        dtpool = ctx.enter_context(
            tc.tile_pool(name="dtree", bufs=cfg["dt_bufs"]))
        lpool = ctx.enter_context(
            tc.tile_pool(name="long", bufs=3))
        spool = ctx.enter_context(
            tc.tile_pool(name="small", bufs=cfg["spool_bufs"]))
        opool = ctx.enter_context(
            tc.tile_pool(name="outp", bufs=cfg["opool_bufs"]))
        bbpool = ctx.enter_context(tc.tile_pool(name="bbp", bufs=3))
        # PSUM: one dedicated single-buffer pool per tile tag; every tile is
        # consumed by an immediately-following copy on a non-PE engine, so
        # cross-batch reuse never creates a queue-order cycle.
        pp_z = ctx.enter_context(tc.tile_pool(name="pp_z", bufs=1, space="PSUM"))
        pp_s3 = ctx.enter_context(tc.tile_pool(name="pp_s3", bufs=1, space="PSUM"))
        pp_ad = ctx.enter_context(tc.tile_pool(name="pp_ad", bufs=1, space="PSUM"))
        pp_g = ctx.enter_context(tc.tile_pool(name="pp_g", bufs=1, space="PSUM"))
        pp_t = ctx.enter_context(tc.tile_pool(name="pp_t", bufs=1, space="PSUM"))

        # ---------------- one-time setup (plain DMAs only) ----------------
        wzT = wpool.tile([128, KC, H], BF16, tag="wzT")
        nc.scalar.dma_start(wzT[:], wzT_d[...])
        w2T = wpool.tile([128, KC, 2], BF16, tag="w2T")
        nc.scalar.dma_start(w2T[:], w2T_d[...])
        bzr = wpool.tile([1, H], BF16, tag="bzr")
        nc.scalar.dma_start(bzr[:], bzt[:, :])
        bar = wpool.tile([1, 1], F32, tag="bar")
        nc.scalar.dma_start(bar[:], bat[:, :])
        m1T = wpool.tile([128, KC, H], BF16, tag="m1T")
        nc.scalar.dma_start(m1T[:], m1T_d[...])
        whzT = wpool.tile([128, KC, H], BF16, tag="whzT")
        nc.scalar.dma_start(whzT[:], whzT_d[...])
        m2r = wpool.tile([E, H], BF16, tag="m2r")
        nc.gpsimd.dma_start(m2r[:], m2_d[...])

        ones_f = wpool.tile([1, 128], F32, tag="ones_f")
        nc.gpsimd.memset(ones_f[:], 1.0)
        ones_b = wpool.tile([1, 128], BF16, tag="ones_b")
        nc.gpsimd.memset(ones_b[:], 1.0)
        id_bf = wpool.tile([128, 128], BF16, tag="id_bf")
        make_identity(nc, id_bf[:])
        negshift = wpool.tile([128, 1], F32, tag="negshift")
        nc.gpsimd.memset(negshift[:], -30.0)

        st = [dict() for _ in range(PB)]

        def stage_a(b):
            """input DMAs for batch b, split across SP/ACT queues."""
            s = st[b]
            bertS = lpool.tile([128, H], F32, tag="bertS")
            s["bertS"] = bertS
            nc.scalar.dma_start(bertS[:], bertS_d[b, :, :])
            bertST = lpool.tile([128, KC, 128], BF16, tag="bertST")
            s["bertST"] = bertST
            nc.scalar.dma_start(bertST[:], bertsT[b, :, :, :])
            dept = dpool.tile([128, L, E], BF16, tag="dept")
            s["dept"] = dept
            nc.sync.dma_start(dept[:, 0:64, :], dep[b, :, 0:64, :])
            nc.scalar.dma_start(dept[:, 64:128, :], dep[b, :, 64:128, :])
            adjt = lpool.tile([128, L], F32, tag="adjt")
            s["adjt"] = adjt
            nc.sync.dma_start(adjt[:], adjf[b, :, :])
            upd = lpool.tile([128, 1], F32, tag="upd")
            s["upd"] = upd
            nc.scalar.dma_start(upd[:], updcol[b, :, :])

        def stage_b(b):
            """scores + softmax for batch b: PE z/s_i/s_j/G, DVE se tree,
            DVE lrelu/mask, ACT exp."""
            s = st[b]
            dept, adjt, bertST = s["dept"], s["adjt"], s["bertST"]

            # zs^T = Wz @ bertS^T + bz: 6 groups packed in one PSUM tile
            p_z = pp_z.tile([128, H], F32, tag="pp_z")
            for hc in range(KC):
                ns = slice(hc * 128, (hc + 1) * 128)
                for kc in range(KC):
                    nc.tensor.matmul(p_z[:, ns], wzT[:, kc, ns],
                                     bertST[:, kc, :],
                                     start=(kc == 0), stop=False)
                nc.tensor.matmul(p_z[:, ns], bzr[0:1, ns], ones_b[:],
                                 start=False, stop=True)
            zsT = lpool.tile([128, KC, 128], BF16, tag="zsT")
            s["zsT"] = zsT
            nc.scalar.copy(zsT[:], p_z[:])
            if b == 0:
                dump("d_zsT", zsT[:])

            # s_i, s_j, score-base packed into one PSUM tile
            p_s3 = pp_s3.tile([128, 384], F32, tag="pp_s3")
            for kc in range(KC):
                nc.tensor.matmul(p_s3[0:1, 0:128], w2T[:, kc, 0:1],
                                 zsT[:, kc, :],
                                 start=(kc == 0), stop=(kc == KC - 1))
            for kc in range(KC):
                nc.tensor.matmul(p_s3[0:1, 128:256], w2T[:, kc, 1:2],
                                 zsT[:, kc, :],
                                 start=(kc == 0), stop=False)
            # fold the scalar ba bias in as a rank-1 (k=1) matmul
            nc.tensor.matmul(p_s3[0:1, 128:256], bar[0:1, 0:1],
                             ones_f[0:1, :], start=False, stop=True)
            si_row = spool.tile([1, 128], F32, tag="si_row")
            nc.scalar.copy(si_row[:], p_s3[0:1, 0:128])
            sjb = spool.tile([1, 128], F32, tag="sjb")
            nc.scalar.copy(sjb[:], p_s3[0:1, 128:256])
            nc.tensor.matmul(p_s3[:, 256:384], si_row[:], ones_f[:],
                             start=True, stop=False)
            nc.tensor.matmul(p_s3[:, 256:384], ones_f[:], sjb[:],
                             start=False, stop=True)
            if b == 0:
                dump("d_si", si_row[:])
                dump("d_sjb", sjb[:])

            # G = zs @ M1 (for temp = attn @ G + ...)
            p_g = pp_g.tile([128, H], F32, tag="pp_g")
            for ns in (slice(0, 512), slice(512, H)):
                for kc in range(KC):
                    nc.tensor.matmul(p_g[:, ns], zsT[:, kc, :],
                                     m1T[:, kc, ns],
                                     start=(kc == 0), stop=(kc == KC - 1))
            gb = lpool.tile([128, H], BF16, tag="gb")
            s["gb"] = gb
            nc.scalar.copy(gb[:], p_g[:])
            if b == 0:
                dump("d_gb", gb[:])

            # s_e = reduce_e(dep'): bf16 2x tree stage + reduce tail
            se1 = sepool.tile([128, L, 32], BF16, tag="se1")
            nc.vector.tensor_tensor(se1[:], dept[:, :, 0:32],
                                    dept[:, :, 32:64], op=OP.add)
            se2 = sepool.tile([128, L, 16], BF16, tag="se2")
            nc.vector.tensor_tensor(se2[:], se1[:, :, 0:16],
                                    se1[:, :, 16:32], op=OP.add)
            se3 = sepool.tile([128, L, 8], BF16, tag="se3")
            nc.vector.tensor_tensor(se3[:], se2[:, :, 0:8],
                                    se2[:, :, 8:16], op=OP.add)
            se = spool.tile([128, L], F32, tag="se")
            nc.vector.tensor_reduce(se[:], se3[:], axis=AX.X, op=OP.add)
            if b == 0:
                dump("d_se", se[:])
                scb_s = spool.tile([128, 128], F32, tag="scb_s")
                nc.vector.tensor_copy(scb_s[:], p_s3[:, 256:384])
                dump("d_scb", scb_s[:])

            # score = lrelu(se + base) via max(x, .01x)
            sadd = spool.tile([128, L], F32, tag="sadd")
            nc.vector.tensor_tensor(sadd[:], se[:], p_s3[:, 256:384],
                                    op=OP.add)
            score = spool.tile([128, L], F32, tag="score")
            nc.vector.scalar_tensor_tensor(
                score[:], sadd[:], 0.01, sadd[:], op0=OP.mult, op1=OP.max)

            # UNNORMALIZED softmax with a fixed shift: exp(score - 30)
            # cannot overflow (score ~ N(0, 2)); adj-masking and the row
            # sum are fused in one tensor_tensor_reduce, seeded with an
            # epsilon so neighbor-less rows give attn 0 instead of NaN.
            # 1/sum is applied later, off the critical path, via scaled
            # ACT copies feeding the transposes.
            ex = spool.tile([128, L], F32, tag="ex")
            nc.scalar.activation(ex[:], score[:], AF.Exp, bias=negshift[:],
                                 scale=1.0)
            u = lpool.tile([128, L], BF16, tag="u")
            s["u"] = u
            nc.vector.tensor_tensor(u[:], ex[:], adjt[:], op=OP.mult)
            sumex = spool.tile([128, 1], F32, tag="sumex")
            nc.vector.tensor_reduce(sumex[:], u[:], axis=AX.X, op=OP.add)
            sume = spool.tile([128, 1], F32, tag="sume")
            nc.vector.tensor_scalar(sume[:], sumex[:], 1e-30, None,
                                    op0=OP.add)
            rec = lpool.tile([128, 1], F32, tag="rec")
            s["rec"] = rec
            nc.vector.reciprocal(rec[:], sume[:])

            # blend precompute: bb = (1-upd) * bertS, off the tail path
            updc = spool.tile([128, 1], F32, tag="updc")
            nc.vector.tensor_scalar(updc[:], s["upd"][:], -1.0, 1.0,
                                    op0=OP.mult, op1=OP.add)
            bb = bbpool.tile([128, H], F32, tag="bb")
            s["bb"] = bb
            nc.vector.tensor_scalar(bb[:], s["bertS"][:], updc[0:128, 0:1],
                                    None, op0=OP.mult)
            if b == 0:
                dump("d_upd", s["upd"][:])

        def stage_c(b):
            """tmp2 = attn * dep' on Pool ([i, e, j] layout)."""
            s = st[b]
            tmp2 = tpool.tile([128, E, L], BF16, tag="ttmp")
            s["tmp2"] = tmp2
            for eh in (slice(0, 32), slice(32, 64)):
                nc.gpsimd.tensor_tensor(
                    tmp2[:, eh, :],
                    s["dept"][:, :, eh].rearrange("p j e -> p e j"),
                    s["u"][:].unsqueeze(1).broadcast_to(
                        [128, eh.stop - eh.start, L]), op=OP.mult)

        def stage_d1(b):
            """D' tree (DVE), 1/sum normalization, transposes."""
            s = st[b]
            tmp2, u, rec = s["tmp2"], s["u"], s["rec"]

            # normalized attn = u * rec (per-partition scale on ACT)
            attnbs = spool.tile([128, L], BF16, tag="attnbs")
            nc.vector.tensor_scalar(attnbs[:], u[:], rec[0:128, 0:1], None,
                                    op0=OP.mult)
            if b == 0:
                dump("d_attn", attnbs[:])
            p_ad = pp_ad.tile([128, 256], BF16, tag="pp_ad")
            nc.tensor.transpose(p_ad[:, 0:128], attnbs[:], id_bf[:])
            attnT = lpool.tile([128, 128], BF16, tag="attnT")
            nc.scalar.copy(attnT[:], p_ad[:, 0:128])

            dvb = spool.tile([128, E], BF16, tag="dvb")
            for hi, eh in enumerate((slice(0, 32), slice(32, 64))):
                dt1h = dtpool.tile([128, 32, 64], BF16, tag=f"dt1h{hi}")
                nc.vector.tensor_tensor(dt1h[:], tmp2[:, eh, 0:64],
                                        tmp2[:, eh, 64:128], op=OP.add)
                dt2h = dtpool.tile([128, 32, 32], BF16, tag=f"dt2h{hi}")
                nc.gpsimd.tensor_tensor(dt2h[:], dt1h[:, :, 0:32],
                                        dt1h[:, :, 32:64], op=OP.add)
                dt3h = dtpool.tile([128, 32, 16], BF16, tag=f"dt3h{hi}")
                nc.vector.tensor_tensor(dt3h[:], dt2h[:, :, 0:16],
                                        dt2h[:, :, 16:32], op=OP.add)
                nc.vector.tensor_reduce(dvb[:, eh], dt3h[:], axis=AX.X,
                                        op=OP.add)
            dvbs = spool.tile([128, E], BF16, tag="dvbs")
            nc.vector.tensor_scalar(dvbs[:], dvb[:], rec[0:128, 0:1], None,
                                    op0=OP.mult)
            if b == 0:
                dump("d_dvec", dvbs[:])

            nc.tensor.transpose(p_ad[0:E, 128:256], dvbs[:], id_bf[:])
            dT = lpool.tile([E, 128], BF16, tag="dT")
            nc.scalar.copy(dT[:], p_ad[0:E, 128:256])
            s["attnT"], s["dT"] = attnT, dT

        def stage_d2(b):
            """temp matmuls (PE), blend, rolled store."""
            s = st[b]
            zsT, gb = s["zsT"], s["gb"]
            attnT, dT = s["attnT"], s["dT"]

            # temp = attn @ G + D' @ M2 + zs @ WhZ^T, per 128-col group
            p_t = pp_t.tile([128, H], F32, tag="pp_t")
            for hc in range(KC):
                ns = slice(hc * 128, (hc + 1) * 128)
                nc.tensor.matmul(p_t[:, ns], attnT[:], gb[:, ns],
                                 start=True, stop=False)
                nc.tensor.matmul(p_t[:, ns], dT[:], m2r[:, ns],
                                 start=False, stop=False)
                for kc in range(KC):
                    nc.tensor.matmul(p_t[:, ns], zsT[:, kc, :],
                                     whzT[:, kc, ns],
                                     start=False, stop=(kc == KC - 1))
            # tb = upd * temp fused into the PSUM->SBUF copy; then add the
            # precomputed (1-upd)*bertS in place and store rolled
            tb = opool.tile([128, H], F32, tag="tb")
            nc.vector.tensor_scalar(tb[:], p_t[:], s["upd"][0:128, 0:1],
                                    None, op0=OP.mult)
            if b == 0:
                dump("d_tempb", tb[:])
            outt = opool.tile([128, H], F32, tag="outt")
            nc.gpsimd.tensor_tensor(outt[:], tb[:], s["bb"][:], op=OP.add)
            nc.sync.dma_start(out[b, 1:128, :], outt[0:127, :])
            nc.sync.dma_start(out[b, 0:1, :], outt[127:128, :])

        # ---- software-pipelined emission ----
        # B(b+1)/C(b+1) ahead of D1(b); D2 one iteration behind so the PE
        # queue always has the next batch's z/G matmuls before temp(b).
        stage_a(0)
        if PB > 1:
            stage_a(1)
        stage_b(0)
        stage_c(0)
        for b in range(PB):
            if b + 2 < PB:
                stage_a(b + 2)
            if b + 1 < PB:
                stage_b(b + 1)
                stage_c(b + 1)
            stage_d1(b)
            if b - 1 >= 0:
                stage_d2(b - 1)
        stage_d2(PB - 1)


def _get_nc():
    if "nc" not in _CACHED:
        _CACHED["nc"] = _build()
    return _CACHED["nc"]


def _chunkT(w):
    """W [rows, K] -> W^T chunk-major [128, K//128, rows] (lhsT layout)."""
    rows, k = w.shape
    return np.ascontiguousarray(
        w.T.reshape(k // 128, 128, rows).transpose(1, 0, 2))


def _prep_in_maps(bert_hidden_states, dep_type_adj, deprel_adj,
                  asp_start, asp_end, Wz, bz, wa, ba, Wf, Wh):
    bf = ml_dtypes.bfloat16
    bert = np.asarray(bert_hidden_states, np.float32)
    wa_f = np.asarray(wa, np.float32)
    wa_e = wa_f[2 * H:]
    # dep' = dep * wa_e fused into the bf16 conversion; 1/wa_e is folded
    # into the D-path weights below so results are unchanged.
    dep = (np.asarray(dep_type_adj, np.float32) * wa_e).astype(bf)
    adjf = np.ascontiguousarray(np.asarray(deprel_adj).astype(np.float32))
    # bertS^T chunk-major per batch: rows shifted by one (the z-roll)
    bs = np.ascontiguousarray(np.roll(bert, -1, axis=1))
    bertsT = np.ascontiguousarray(
        bs.transpose(0, 2, 1).reshape(B, KC, 128, L).transpose(0, 2, 1, 3)
    ).astype(bf)
    pos = np.arange(L, dtype=np.float32)
    s_ = np.asarray(asp_start).astype(np.float32)[:, None]
    e_ = np.asarray(asp_end).astype(np.float32)[:, None]
    anyn = (np.asarray(deprel_adj) > 0).any(-1)
    upd_full = (((pos[None, :] >= s_) & (pos[None, :] <= e_)) & anyn).astype(
        np.float32)[:, :, None]

    Wz = np.asarray(Wz, np.float32)
    Wf = np.asarray(Wf, np.float32)
    Wh = np.asarray(Wh, np.float32)
    WhN = Wh[:, :H]   # temp uses nbr @ WhN^T
    WhZ = Wh[:, H:]
    WfZ = Wf[:, :H]
    WfE = Wf[:, H:] / wa_e[None, :]   # compensate host dep*wa_e scaling
    # M1 = WfZ^T @ WhN^T  (so attn @ (zs @ M1) == (attn @ zs @ WfZ^T) @ WhN^T)
    M1 = WfZ.T @ WhN.T
    # M2 = WfE'^T @ WhN^T  (so D' @ M2 == (D @ WfE^T) @ WhN^T)
    M2 = WfE.T @ WhN.T
    wzT = _chunkT(Wz).astype(bf)
    m1T = _chunkT(M1.T).astype(bf)    # lhsT layout for zs @ M1
    whzT = _chunkT(WhZ).astype(bf)
    m2b = np.ascontiguousarray(M2).astype(bf)
    w2T = _chunkT(wa_f[:2 * H].reshape(2, H)).astype(bf)
    bzb = np.asarray(bz, np.float32)[None, :].astype(bf)
    bab = np.asarray(ba, np.float32).reshape(1, 1)

    in_maps = []
    for c in range(NCORES):
        s = slice(c * PB, (c + 1) * PB)
        in_maps.append(dict(
            bertS_d=bs[s], bertsT=np.ascontiguousarray(bertsT[s]),
            dep=dep[s], adjf=adjf[s],
            updcol=np.ascontiguousarray(upd_full[s]),
            wzT=wzT, m1T=m1T, whzT=whzT, m2=m2b, w2T=w2T,
            bzt=bzb, bat=bab,
        ))
    return in_maps


def kernel(bert_hidden_states, dep_type_adj, deprel_adj, asp_start, asp_end,
           Wz, bz, wa, ba, Wf, Wh):
    from concourse.bass_utils import run_bass_kernel_spmd

    in_maps = _prep_in_maps(bert_hidden_states, dep_type_adj, deprel_adj,
                            asp_start, asp_end, Wz, bz, wa, ba, Wf, Wh)
    nc = _get_nc()
    res = run_bass_kernel_spmd(nc, in_maps, core_ids=list(range(NCORES)),
                               trace=bool(_CACHED.get("trace")))
    _CACHED["last_results"] = res
    outs = [res.results[c]["out"] for c in range(NCORES)]
    return np.concatenate(outs, axis=0).astype(np.float32)


# revision 27
# speedup vs baseline: 1.0029x; 1.0029x over previous
"""Trainium2 Bass kernel for AspectNeighborAttention (gnn_message_passing).

Pure data-parallel over batch: 32 batches -> 8 NeuronCores x 4 batches.
All weights replicated, host-converted to bf16 and host-PRE-TRANSPOSED into
the chunk-major [128, KC, *] lhsT/rhs layouts the TensorEngine wants.

Key tricks vs the naive formulation:
 - dep is host-scaled by wa_e and stored bf16 ("dep'"); the matching 1/wa_e
   is folded into Wf[:, H:]. s_e then needs only a reduce over e, and the
   D-path result D' = sum_j attn*dep' feeds the rescaled weights.
 - weight folding: temp = nbr @ WhN^T + zs @ WhZ^T with
   nbr = attn @ (zs @ WfZ^T) + D @ WfE^T collapses via host-precomputed
   M1 = WfZ^T WhN^T and M2 = WfE'^T WhN^T to
   temp = attn @ (zs @ M1) + D' @ M2 + zs @ WhZ^T  (no nbr intermediate).
 - the big e/j reductions run as bf16 tensor_tensor ADD trees (2x DVE mode)
   with a short tensor_reduce tail; monolithic reduces get no 2x.
 - the attn * dep' multiply runs entirely on GpSimd (Pool), balancing DVE.
 - leaky-relu is computed as max(x, 0.01*x) on DVE so the ACT engine keeps
   only the Exp table loaded (no per-batch activation-table swaps).
 - input DMAs are split across the SP and Activation HWDGE queues.
 - the batch loop is software-pipelined: emission order B(b+1), C(b+1),
   D(b) keeps every engine's in-order queue free of cross-batch stalls
   (B = scores/softmax, C = Pool multiply, D = D-tree + output matmuls).

The roll(z,-1)/roll(out,+1) pair is handled purely with shifted-row DMAs.
"""

import sys

for _p in ("/opt/trn_rl_repo",):
    if _p not in sys.path:
        sys.path.insert(0, _p)

import os
import numpy as np
import ml_dtypes

import concourse.bass as bass
import concourse.bacc as bacc_mod
import concourse.mybir as mybir
import concourse.tile as tile
from concourse.masks import make_identity

B, L, H, E = 32, 128, 768, 64
NCORES = 8
PB = B // NCORES  # batches per core
KC = H // 128     # 6 k-chunks
F32 = mybir.dt.float32
BF16 = mybir.dt.bfloat16
AF = mybir.ActivationFunctionType
OP = mybir.AluOpType
AX = mybir.AxisListType
MASK_SHIFT = 10000.0  # additive mask offset (see score masking)

_CACHED = {}

CFG = dict(
    dep_bufs=int(os.environ.get("K_DEP_BUFS", 3)),
    ttmp_bufs=int(os.environ.get("K_TTMP_BUFS", 2)),
    se_bufs=int(os.environ.get("K_SE_BUFS", 1)),
    dt_bufs=int(os.environ.get("K_DT_BUFS", 1)),
    spool_bufs=int(os.environ.get("K_SPOOL_BUFS", 2)),
    opool_bufs=int(os.environ.get("K_OPOOL_BUFS", 2)),
)


def _build(debug=False):
    nc = bacc_mod.Bacc("TRN2", target_bir_lowering=False, debug=False,
                       num_devices=NCORES)

    bertS_d = nc.dram_tensor("bertS_d", [PB, L, H], F32,
                             kind="ExternalInput")
    bertsT = nc.dram_tensor("bertsT", [PB, 128, KC, 128], BF16,
                            kind="ExternalInput")
    dep = nc.dram_tensor("dep", [PB, L, L, E], BF16, kind="ExternalInput")
    adjf = nc.dram_tensor("adjf", [PB, L, L], F32, kind="ExternalInput")
    updcol = nc.dram_tensor("updcol", [PB, 128, 1], F32,
                              kind="ExternalInput")
    wzT_d = nc.dram_tensor("wzT", [128, KC, H], BF16, kind="ExternalInput")
    m1T_d = nc.dram_tensor("m1T", [128, KC, H], BF16, kind="ExternalInput")
    whzT_d = nc.dram_tensor("whzT", [128, KC, H], BF16, kind="ExternalInput")
    m2_d = nc.dram_tensor("m2", [E, H], BF16, kind="ExternalInput")
    w2T_d = nc.dram_tensor("w2T", [128, KC, 2], BF16, kind="ExternalInput")
    bzt = nc.dram_tensor("bzt", [1, H], BF16, kind="ExternalInput")
    bat = nc.dram_tensor("bat", [1, 1], F32, kind="ExternalInput")
    out = nc.dram_tensor("out", [PB, L, H], F32, kind="ExternalOutput")

    dbg = {}
    if debug:
        for nm, shape, dt in [
            ("d_zsT", [128, KC, 128], BF16), ("d_si", [1, 128], BF16),
            ("d_sjb", [1, 128], BF16), ("d_se", [128, L], F32),
            ("d_masked", [128, L], F32), ("d_attn", [128, L], BF16),
            ("d_dvec", [128, E], BF16), ("d_gb", [128, H], BF16),
            ("d_tempb", [128, H], F32),
            ("d_upd", [128, 1], F32), ("d_scb", [128, 128], F32),
        ]:
            dbg[nm] = nc.dram_tensor(nm, shape, dt, kind="ExternalOutput")
    with tile.TileContext(nc) as tc:
        with nc.allow_low_precision("bf16 softmax/D path, 2e-2 rel-err gate"):
            _body(tc, nc, bertS_d, bertsT, dep, adjf, updcol, wzT_d, m1T_d,
                  whzT_d, m2_d, w2T_d, bzt, bat, out, dbg)
    nc.compile()
    return nc


def _body(tc, nc, bertS_d, bertsT, dep, adjf, updcol, wzT_d, m1T_d,
          whzT_d, m2_d, w2T_d, bzt, bat, out, dbg=None):
    def dump(name, ap):
        if dbg and name in dbg:
            nc.sync.dma_start(dbg[name][...], ap)
    import contextlib
    cfg = CFG
    ctx = contextlib.ExitStack()
    with ctx:
        wpool = ctx.enter_context(tc.tile_pool(name="weights", bufs=1))
        dpool = ctx.enter_context(
            tc.tile_pool(name="dep", bufs=cfg["dep_bufs"]))
        tpool = ctx.enter_context(
            tc.tile_pool(name="ttmp", bufs=cfg["ttmp_bufs"]))
        sepool = ctx.enter_context(
            tc.tile_pool(name="setree", bufs=cfg["se_bufs"]))
        dtpool = ctx.enter_context(
            tc.tile_pool(name="dtree", bufs=cfg["dt_bufs"]))
        lpool = ctx.enter_context(
            tc.tile_pool(name="long", bufs=3))
        spool = ctx.enter_context(
            tc.tile_pool(name="small", bufs=cfg["spool_bufs"]))
        opool = ctx.enter_context(
            tc.tile_pool(name="outp", bufs=cfg["opool_bufs"]))
        bbpool = ctx.enter_context(tc.tile_pool(name="bbp", bufs=3))
        # PSUM: one dedicated single-buffer pool per tile tag; every tile is
        # consumed by an immediately-following copy on a non-PE engine, so
        # cross-batch reuse never creates a queue-order cycle.
        pp_z = ctx.enter_context(tc.tile_pool(name="pp_z", bufs=1, space="PSUM"))
        pp_s3 = ctx.enter_context(tc.tile_pool(name="pp_s3", bufs=1, space="PSUM"))
        pp_ad = ctx.enter_context(tc.tile_pool(name="pp_ad", bufs=1, space="PSUM"))
        pp_g = ctx.enter_context(tc.tile_pool(name="pp_g", bufs=1, space="PSUM"))
        pp_t = ctx.enter_context(tc.tile_pool(name="pp_t", bufs=1, space="PSUM"))

        # ---------------- one-time setup (plain DMAs only) ----------------
        wzT = wpool.tile([128, KC, H], BF16, tag="wzT")
        nc.scalar.dma_start(wzT[:], wzT_d[...])
        w2T = wpool.tile([128, KC, 2], BF16, tag="w2T")
        nc.scalar.dma_start(w2T[:], w2T_d[...])
        bzr = wpool.tile([1, H], BF16, tag="bzr")
        nc.scalar.dma_start(bzr[:], bzt[:, :])
        bar = wpool.tile([1, 1], F32, tag="bar")
        nc.scalar.dma_start(bar[:], bat[:, :])
        m1T = wpool.tile([128, KC, H], BF16, tag="m1T")
        nc.scalar.dma_start(m1T[:], m1T_d[...])
        whzT = wpool.tile([128, KC, H], BF16, tag="whzT")
        nc.scalar.dma_start(whzT[:], whzT_d[...])
        m2r = wpool.tile([E, H], BF16, tag="m2r")
        nc.gpsimd.dma_start(m2r[:], m2_d[...])

        ones_f = wpool.tile([1, 128], F32, tag="ones_f")
        nc.gpsimd.memset(ones_f[:], 1.0)
        ones_b = wpool.tile([1, 128], BF16, tag="ones_b")
        nc.gpsimd.memset(ones_b[:], 1.0)
        id_bf = wpool.tile([128, 128], BF16, tag="id_bf")
        make_identity(nc, id_bf[:])
        negshift = wpool.tile([128, 1], F32, tag="negshift")
        nc.gpsimd.memset(negshift[:], -30.0)

        st = [dict() for _ in range(PB)]

        def stage_a(b):
            """input DMAs for batch b, split across SP/ACT queues."""
            s = st[b]
            bertS = lpool.tile([128, H], F32, tag="bertS")
            s["bertS"] = bertS
            nc.scalar.dma_start(bertS[:], bertS_d[b, :, :])
            bertST = lpool.tile([128, KC, 128], BF16, tag="bertST")
            s["bertST"] = bertST
            nc.scalar.dma_start(bertST[:], bertsT[b, :, :, :])
            dept = dpool.tile([128, L, E], BF16, tag="dept")
            s["dept"] = dept
            nc.sync.dma_start(dept[:, 0:64, :], dep[b, :, 0:64, :])
            nc.scalar.dma_start(dept[:, 64:128, :], dep[b, :, 64:128, :])
            adjt = lpool.tile([128, L], F32, tag="adjt")
            s["adjt"] = adjt
            nc.sync.dma_start(adjt[:], adjf[b, :, :])
            upd = lpool.tile([128, 1], F32, tag="upd")
            s["upd"] = upd
            nc.scalar.dma_start(upd[:], updcol[b, :, :])

        def stage_b(b):
            """scores + softmax for batch b: PE z/s_i/s_j/G, DVE se tree,
            DVE lrelu/mask, ACT exp."""
            s = st[b]
            dept, adjt, bertST = s["dept"], s["adjt"], s["bertST"]

            # zs^T = Wz @ bertS^T + bz: 6 groups packed in one PSUM tile
            p_z = pp_z.tile([128, H], F32, tag="pp_z")
            for hc in range(KC):
                ns = slice(hc * 128, (hc + 1) * 128)
                for kc in range(KC):
                    nc.tensor.matmul(p_z[:, ns], wzT[:, kc, ns],
                                     bertST[:, kc, :],
                                     start=(kc == 0), stop=False)
                nc.tensor.matmul(p_z[:, ns], bzr[0:1, ns], ones_b[:],
                                 start=False, stop=True)
            zsT = lpool.tile([128, KC, 128], BF16, tag="zsT")
            s["zsT"] = zsT
            nc.scalar.copy(zsT[:], p_z[:])
            if b == 0:
                dump("d_zsT", zsT[:])

            # s_i, s_j, score-base packed into one PSUM tile
            p_s3 = pp_s3.tile([128, 384], F32, tag="pp_s3")
            for kc in range(KC):
                nc.tensor.matmul(p_s3[0:1, 0:128], w2T[:, kc, 0:1],
                                 zsT[:, kc, :],
                                 start=(kc == 0), stop=(kc == KC - 1))
            for kc in range(KC):
                nc.tensor.matmul(p_s3[0:1, 128:256], w2T[:, kc, 1:2],
                                 zsT[:, kc, :],
                                 start=(kc == 0), stop=False)
            # fold the scalar ba bias in as a rank-1 (k=1) matmul
            nc.tensor.matmul(p_s3[0:1, 128:256], bar[0:1, 0:1],
                             ones_f[0:1, :], start=False, stop=True)
            si_row = spool.tile([1, 128], F32, tag="si_row")
            nc.scalar.copy(si_row[:], p_s3[0:1, 0:128])
            sjb = spool.tile([1, 128], F32, tag="sjb")
            nc.scalar.copy(sjb[:], p_s3[0:1, 128:256])
            nc.tensor.matmul(p_s3[:, 256:384], si_row[:], ones_f[:],
                             start=True, stop=False)
            nc.tensor.matmul(p_s3[:, 256:384], ones_f[:], sjb[:],
                             start=False, stop=True)
            if b == 0:
                dump("d_si", si_row[:])
                dump("d_sjb", sjb[:])

            # G = zs @ M1 (for temp = attn @ G + ...)
            p_g = pp_g.tile([128, H], F32, tag="pp_g")
            for ns in (slice(0, 512), slice(512, H)):
                for kc in range(KC):
                    nc.tensor.matmul(p_g[:, ns], zsT[:, kc, :],
                                     m1T[:, kc, ns],
                                     start=(kc == 0), stop=(kc == KC - 1))
            gb = lpool.tile([128, H], BF16, tag="gb")
            s["gb"] = gb
            nc.scalar.copy(gb[:], p_g[:])
            if b == 0:
                dump("d_gb", gb[:])

            # s_e = reduce_e(dep'): bf16 2x tree stage + reduce tail
            se1 = sepool.tile([128, L, 32], BF16, tag="se1")
            nc.vector.tensor_tensor(se1[:], dept[:, :, 0:32],
                                    dept[:, :, 32:64], op=OP.add)
            se2 = sepool.tile([128, L, 16], BF16, tag="se2")
            nc.vector.tensor_tensor(se2[:], se1[:, :, 0:16],
                                    se1[:, :, 16:32], op=OP.add)
            se3 = sepool.tile([128, L, 8], BF16, tag="se3")
            nc.vector.tensor_tensor(se3[:], se2[:, :, 0:8],
                                    se2[:, :, 8:16], op=OP.add)
            se = spool.tile([128, L], F32, tag="se")
            nc.vector.tensor_reduce(se[:], se3[:], axis=AX.X, op=OP.add)
            if b == 0:
                dump("d_se", se[:])
                scb_s = spool.tile([128, 128], F32, tag="scb_s")
                nc.vector.tensor_copy(scb_s[:], p_s3[:, 256:384])
                dump("d_scb", scb_s[:])

            # score = lrelu(se + base) via max(x, .01x)
            sadd = spool.tile([128, L], F32, tag="sadd")
            nc.vector.tensor_tensor(sadd[:], se[:], p_s3[:, 256:384],
                                    op=OP.add)
            score = spool.tile([128, L], F32, tag="score")
            nc.vector.scalar_tensor_tensor(
                score[:], sadd[:], 0.01, sadd[:], op0=OP.mult, op1=OP.max)

            # UNNORMALIZED softmax with a fixed shift: exp(score - 30)
            # cannot overflow (score ~ N(0, 2)); adj-masking and the row
            # sum are fused in one tensor_tensor_reduce, seeded with an
            # epsilon so neighbor-less rows give attn 0 instead of NaN.
            # 1/sum is applied later, off the critical path, via scaled
            # ACT copies feeding the transposes.
            ex = spool.tile([128, L], F32, tag="ex")
            nc.scalar.activation(ex[:], score[:], AF.Exp, bias=negshift[:],
                                 scale=1.0)
            u = lpool.tile([128, L], BF16, tag="u")
            s["u"] = u
            nc.vector.tensor_tensor(u[:], ex[:], adjt[:], op=OP.mult)
            sumex = spool.tile([128, 1], F32, tag="sumex")
            nc.vector.tensor_reduce(sumex[:], u[:], axis=AX.X, op=OP.add)
            sume = spool.tile([128, 1], F32, tag="sume")
            nc.vector.tensor_scalar(sume[:], sumex[:], 1e-30, None,
                                    op0=OP.add)
            rec = lpool.tile([128, 1], F32, tag="rec")
            s["rec"] = rec
            nc.vector.reciprocal(rec[:], sume[:])

            # blend precompute: bb = (1-upd) * bertS, off the tail path
            updc = spool.tile([128, 1], F32, tag="updc")
            nc.vector.tensor_scalar(updc[:], s["upd"][:], -1.0, 1.0,
                                    op0=OP.mult, op1=OP.add)
            bb = bbpool.tile([128, H], F32, tag="bb")
            s["bb"] = bb
            nc.vector.tensor_scalar(bb[:], s["bertS"][:], updc[0:128, 0:1],
                                    None, op0=OP.mult)
            if b == 0:
                dump("d_upd", s["upd"][:])

        def stage_c(b):
            """tmp2 = attn * dep' on Pool ([i, e, j] layout)."""
            s = st[b]
            tmp2 = tpool.tile([128, E, L], BF16, tag="ttmp")
            s["tmp2"] = tmp2
            for eh in (slice(0, 32), slice(32, 64)):
                nc.gpsimd.tensor_tensor(
                    tmp2[:, eh, :],
                    s["dept"][:, :, eh].rearrange("p j e -> p e j"),
                    s["u"][:].unsqueeze(1).broadcast_to(
                        [128, eh.stop - eh.start, L]), op=OP.mult)

        def stage_d1(b):
            """D' tree (DVE), 1/sum normalization, transposes."""
            s = st[b]
            tmp2, u, rec = s["tmp2"], s["u"], s["rec"]

            # normalized attn = u * rec (per-partition scale on ACT)
            attnbs = spool.tile([128, L], BF16, tag="attnbs")
            nc.vector.tensor_scalar(attnbs[:], u[:], rec[0:128, 0:1], None,
                                    op0=OP.mult)
            if b == 0:
                dump("d_attn", attnbs[:])
            p_ad = pp_ad.tile([128, 256], BF16, tag="pp_ad")
            nc.tensor.transpose(p_ad[:, 0:128], attnbs[:], id_bf[:])
            attnT = lpool.tile([128, 128], BF16, tag="attnT")
            nc.scalar.copy(attnT[:], p_ad[:, 0:128])

            dvb = spool.tile([128, E], BF16, tag="dvb")
            for hi, eh in enumerate((slice(0, 32), slice(32, 64))):
                dt1h = dtpool.tile([128, 32, 64], BF16, tag=f"dt1h{hi}")
                nc.vector.tensor_tensor(dt1h[:], tmp2[:, eh, 0:64],
                                        tmp2[:, eh, 64:128], op=OP.add)
                dt2h = dtpool.tile([128, 32, 32], BF16, tag=f"dt2h{hi}")
                nc.gpsimd.tensor_tensor(dt2h[:], dt1h[:, :, 0:32],
                                        dt1h[:, :, 32:64], op=OP.add)
                dt3h = dtpool.tile([128, 32, 16], BF16, tag=f"dt3h{hi}")
                nc.vector.tensor_tensor(dt3h[:], dt2h[:, :, 0:16],
                                        dt2h[:, :, 16:32], op=OP.add)
                nc.vector.tensor_reduce(dvb[:, eh], dt3h[:], axis=AX.X,
                                        op=OP.add)
            dvbs = spool.tile([128, E], BF16, tag="dvbs")
            nc.vector.tensor_scalar(dvbs[:], dvb[:], rec[0:128, 0:1], None,
                                    op0=OP.mult)
            if b == 0:
                dump("d_dvec", dvbs[:])

            nc.tensor.transpose(p_ad[0:E, 128:256], dvbs[:], id_bf[:])
            dT = lpool.tile([E, 128], BF16, tag="dT")
            nc.scalar.copy(dT[:], p_ad[0:E, 128:256])
            s["attnT"], s["dT"] = attnT, dT

        def stage_d2(b):
            """temp matmuls (PE), blend, rolled store."""
            s = st[b]
            zsT, gb = s["zsT"], s["gb"]
            attnT, dT = s["attnT"], s["dT"]

            # temp = attn @ G + D' @ M2 + zs @ WhZ^T, per 128-col group
            p_t = pp_t.tile([128, H], F32, tag="pp_t")
            for hc in range(KC):
                ns = slice(hc * 128, (hc + 1) * 128)
                nc.tensor.matmul(p_t[:, ns], attnT[:], gb[:, ns],
                                 start=True, stop=False)
                nc.tensor.matmul(p_t[:, ns], dT[:], m2r[:, ns],
                                 start=False, stop=False)
                for kc in range(KC):
                    nc.tensor.matmul(p_t[:, ns], zsT[:, kc, :],
                                     whzT[:, kc, ns],
                                     start=False, stop=(kc == KC - 1))
            # tb = upd * temp fused into the PSUM->SBUF copy; then add the
            # precomputed (1-upd)*bertS in place and store rolled
            tb = opool.tile([128, H], F32, tag="tb")
            nc.vector.tensor_scalar(tb[:], p_t[:], s["upd"][0:128, 0:1],
                                    None, op0=OP.mult)
            if b == 0:
                dump("d_tempb", tb[:])
            outt = opool.tile([128, H], F32, tag="outt")
            nc.gpsimd.tensor_tensor(outt[:], tb[:], s["bb"][:], op=OP.add)
            nc.sync.dma_start(out[b, 1:128, :], outt[0:127, :])
            nc.sync.dma_start(out[b, 0:1, :], outt[127:128, :])

        # ---- software-pipelined emission ----
        # B(b+1)/C(b+1) ahead of D1(b); D2 one iteration behind so the PE
        # queue always has the next batch's z/G matmuls before temp(b).
        stage_a(0)
        if PB > 1:
            stage_a(1)
        stage_b(0)
        stage_c(0)
        for b in range(PB):
            if b + 2 < PB:
                stage_a(b + 2)
            if b + 1 < PB:
                stage_b(b + 1)
                stage_c(b + 1)
            stage_d1(b)
            if b - 1 >= 0:
                stage_d2(b - 1)
        stage_d2(PB - 1)


def _get_nc():
    if "nc" not in _CACHED:
        _CACHED["nc"] = _build()
    return _CACHED["nc"]


def _chunkT(w):
    """W [rows, K] -> W^T chunk-major [128, K//128, rows] (lhsT layout)."""
    rows, k = w.shape
    return np.ascontiguousarray(
        w.T.reshape(k // 128, 128, rows).transpose(1, 0, 2))


def _prep_in_maps(bert_hidden_states, dep_type_adj, deprel_adj,
                  asp_start, asp_end, Wz, bz, wa, ba, Wf, Wh):
    bf = ml_dtypes.bfloat16
    bert = np.asarray(bert_hidden_states, np.float32)
    wa_f = np.asarray(wa, np.float32)
    wa_e = wa_f[2 * H:]
    # dep' = dep * wa_e fused into the bf16 conversion; 1/wa_e is folded
    # into the D-path weights below so results are unchanged.
    dep = (np.asarray(dep_type_adj, np.float32) * wa_e).astype(bf)
    adjf = np.ascontiguousarray(np.asarray(deprel_adj).astype(np.float32))
    # bertS^T chunk-major per batch: rows shifted by one (the z-roll)
    bs = np.ascontiguousarray(np.roll(bert, -1, axis=1))
    bertsT = np.ascontiguousarray(
        bs.transpose(0, 2, 1).reshape(B, KC, 128, L).transpose(0, 2, 1, 3)
    ).astype(bf)
    pos = np.arange(L, dtype=np.float32)
    s_ = np.asarray(asp_start).astype(np.float32)[:, None]
    e_ = np.asarray(asp_end).astype(np.float32)[:, None]
    anyn = (np.asarray(deprel_adj) > 0).any(-1)
    upd_full = (((pos[None, :] >= s_) & (pos[None, :] <= e_)) & anyn).astype(
        np.float32)[:, :, None]

    Wz = np.asarray(Wz, np.float32)
    Wf = np.asarray(Wf, np.float32)
    Wh = np.asarray(Wh, np.float32)
    WhN = Wh[:, :H]   # temp uses nbr @ WhN^T
    WhZ = Wh[:, H:]
    WfZ = Wf[:, :H]
    WfE = Wf[:, H:] / wa_e[None, :]   # compensate host dep*wa_e scaling
    # M1 = WfZ^T @ WhN^T  (so attn @ (zs @ M1) == (attn @ zs @ WfZ^T) @ WhN^T)
    M1 = WfZ.T @ WhN.T
    # M2 = WfE'^T @ WhN^T  (so D' @ M2 == (D @ WfE^T) @ WhN^T)
    M2 = WfE.T @ WhN.T
    wzT = _chunkT(Wz).astype(bf)
    m1T = _chunkT(M1.T).astype(bf)    # lhsT layout for zs @ M1
    whzT = _chunkT(WhZ).astype(bf)
    m2b = np.ascontiguousarray(M2).astype(bf)
    w2T = _chunkT(wa_f[:2 * H].reshape(2, H)).astype(bf)
    bzb = np.asarray(bz, np.float32)[None, :].astype(bf)
    bab = np.asarray(ba, np.float32).reshape(1, 1)

    in_maps = []
    for c in range(NCORES):
        s = slice(c * PB, (c + 1) * PB)
        in_maps.append(dict(
            bertS_d=bs[s], bertsT=np.ascontiguousarray(bertsT[s]),
            dep=dep[s], adjf=adjf[s],
            updcol=np.ascontiguousarray(upd_full[s]),
            wzT=wzT, m1T=m1T, whzT=whzT, m2=m2b, w2T=w2T,
            bzt=bzb, bat=bab,
        ))
    return in_maps


def kernel(bert_hidden_states, dep_type_adj, deprel_adj, asp_start, asp_end,
           Wz, bz, wa, ba, Wf, Wh):
    from concourse.bass_utils import run_bass_kernel_spmd

    in_maps = _prep_in_maps(bert_hidden_states, dep_type_adj, deprel_adj,
                            asp_start, asp_end, Wz, bz, wa, ba, Wf, Wh)
    nc = _get_nc()
    res = run_bass_kernel_spmd(nc, in_maps, core_ids=list(range(NCORES)),
                               trace=bool(_CACHED.get("trace")))
    _CACHED["last_results"] = res
    outs = [res.results[c]["out"] for c in range(NCORES)]
    return np.concatenate(outs, axis=0).astype(np.float32)
